# revision 1
# baseline (speedup 1.0000x reference)
"""BioBertNerAdvanced v2 kernel — 8 TRN2 cores, pure data parallel (8 batches/core).

Key structure vs v1:
  - word_bert scatter -> ONE dma_gather (transpose) per batch straight from
    DRAM bert rows (first-subword index per word, host-computed; missing
    words hit an appended zero row). Kills the one-hot scatter matmuls,
    their PSUM evac, and half the bert HBM traffic.
  - char conv on a unified 4-tap window: t=0..15 main positions + t=16 edge
    position. Pass A = 128 filter rows [k4(50)|k2a(28)|k3(78:128)] so the
    edge merge is one contiguous [0:78] slice; pass B = 22 rows (k2b)
    packed 2x at 64-partition pitch.
  - conv matmuls in fp8e4 DoubleRow (2 k-tiles of 60 partitions = 120-row
    contraction at 0.5 cyc/col).
  - max-over-time: pass A via DVE tensor_reduce straight off PSUM (single
    PSUM input is legal; tt with two PSUM inputs is not); pass B via
    ScalarE relu+bias evac then a Pool bf16 tt tree (GPSIMD cannot read
    PSUM). relu(max+b) == max(relu(+b)) makes the orders equivalent.
  - char embedding one-hot matmul in bf16 (DVE 4x is_equal), psum evacuated
    to fp8 X mostly on ScalarE.
  - two-stage software pipeline: frontend(b+1) [gather/one-hot/embed/X/pos]
    overlaps backend(b) [conv/drains/MLP]; bulk DMAs on SP/Act HWDGE
    queues; constants + preloads on SWDGE during warmup.
"""

import numpy as np
import ml_dtypes

import bass_rust
import concourse.bass as bass
import concourse.tile as tile
from concourse import mybir, library_config
from concourse.bass_utils import run_bass_kernel_spmd
from concourse.library_overlay import lower_extended_insts
from concourse.tile import ScopedClock


def _split_drain_and_barrier(self, tick_clock, wait_clock):
    """TileContext tail-drain emits one instruction with a sem wait per
    in-flight proc; walrus rejects >2 sync waits on one instruction. Spread
    the waits over SP nops (program order on SP makes the drain safe)."""
    gc = tick_clock.global_clock
    vals = list(gc)
    for i, v in enumerate(vals):
        if v > 0:
            part = [0] * len(vals)
            part[i] = v
            nop = self.nc.sync.nop()
            wait_clock.add_sem_waits(
                nop.ins, ScopedClock({None: bass_rust.VectorClock(part)})
            )
    drain_inst = self.nc.sync.drain()
    wait_clock.add_sem_waits(
        drain_inst.ins, ScopedClock({None: gc}), cur_clock=ScopedClock({None: gc})
    )
    self.nc.all_engine_barrier()
    assert self.sems is not None
    popped = self.nc._tile_sem_poison_stack.pop()
    assert popped is self._sem_poison
    self.nc.clear_and_free_semaphores(list(self.sems.allocated().values()))
    self.nc.all_engine_barrier()


tile.TileContext._drain_and_barrier = _split_drain_and_barrier

WAIT_LIMIT = 1


def _split_excess_waits(nc):
    """Walrus rejects instructions carrying more than WAIT_LIMIT sync waits.
    Spill the excess onto same-engine nops inserted immediately before the
    instruction (engine FIFO order makes the waits equivalent)."""
    for bb in nc.main_func.blocks:
        insts = bb.instructions
        out = []
        for ins in insts:
            si = ins.sync_info
            ow = list(si.on_wait) if si is not None and si.on_wait else []
            if len(ow) > WAIT_LIMIT:
                excess, keep = ow[:-WAIT_LIMIT], ow[-WAIT_LIMIT:]
                for i in range(0, len(excess), WAIT_LIMIT):
                    grp = excess[i:i + WAIT_LIMIT]
                    nop = nc.engines[ins.engine].nop(nofuse=True)
                    for bb2 in nc.main_func.blocks:
                        if bb2.instructions and bb2.instructions[-1] is nop.ins:
                            bb2.instructions.pop()
                            break
                    nop.ins.sync_info = mybir.SyncInfo(on_wait=grp, on_update=[])
                    out.append(nop.ins)
                si.on_wait = keep
            out.append(ins)
        insts[:] = out


# problem dims
B, S, H = 64, 512, 768
W, LC = 256, 16
CV, CE, NF = 256, 30, 50
NPOS, PEM = 20, 25
HID, NL = 256, 9
N_CORES = 8
BB = B // N_CORES
TP = 20                    # padded char pitch per word (2 + 16 + 2)
NP = W * TP                # char stream length per batch (5120)
HCH = 6                    # h-chunks of 128
NA = 128                   # pass-A rows: k4(0:50) k2a(50:78) k3(78:128)
NB = NF - 28               # pass-B filters (k2b, 22)
NBW = 64                   # B stationary width: (tg, f) pairs 2x22 padded to 64

F32 = mybir.dt.float32
BF16 = mybir.dt.bfloat16
FP8 = mybir.dt.float8e4
I16 = mybir.dt.int16

RELU = mybir.ActivationFunctionType.Relu
IDENT = mybir.ActivationFunctionType.Identity
MAX = mybir.AluOpType.max
ADD = mybir.AluOpType.add
ISEQ = mybir.AluOpType.is_equal
AXX = mybir.AxisListType.X
DR = mybir.MatmulPerfMode.DoubleRow


def build_nc():
    nc = bass.Bass("TRN2", target_bir_lowering=False, debug=False)

    bert_d = nc.dram_tensor("bert", [BB, S + 1, H], BF16, kind="ExternalInput")
    wbidx_d = nc.dram_tensor("wbidx", [128, BB, W // 16], I16, kind="ExternalInput")
    cflat_d = nc.dram_tensor("cflat", [1, BB * NP], BF16, kind="ExternalInput")
    pflat_d = nc.dram_tensor("pflat", [1, BB * W], BF16, kind="ExternalInput")
    iotav_d = nc.dram_tensor("iotav", [128, 2], F32, kind="ExternalInput")
    iota20_d = nc.dram_tensor("iota20", [NPOS, 1], F32, kind="ExternalInput")
    cemb_d = nc.dram_tensor("cemb", [128, 2, CE], BF16, kind="ExternalInput")
    petab_d = nc.dram_tensor("petab", [NPOS, PEM], BF16, kind="ExternalInput")
    cwa_d = nc.dram_tensor("cwa", [60, 2, NA], FP8, kind="ExternalInput")
    cwb_d = nc.dram_tensor("cwb", [60, 2, NBW], FP8, kind="ExternalInput")
    ba_d = nc.dram_tensor("ba", [NA, 1], F32, kind="ExternalInput")
    bb_d = nc.dram_tensor("bb", [NB, 1], F32, kind="ExternalInput")
    whb_d = nc.dram_tensor("whb", [128, HCH, HID], BF16, kind="ExternalInput")
    whca_d = nc.dram_tensor("whca", [NA, HID], BF16, kind="ExternalInput")
    whcb_d = nc.dram_tensor("whcb", [NB, HID], BF16, kind="ExternalInput")
    whp_d = nc.dram_tensor("whp", [PEM, HID], BF16, kind="ExternalInput")
    bh_d = nc.dram_tensor("bh", [128, 2], F32, kind="ExternalInput")
    wc_d = nc.dram_tensor("wc", [128, 2, NL], BF16, kind="ExternalInput")
    bc_d = nc.dram_tensor("bc", [NL, 1], F32, kind="ExternalInput")
    id_d = nc.dram_tensor("ident", [NL, NL], F32, kind="ExternalInput")
    out_d = nc.dram_tensor("out", [BB, W, NL], F32, kind="ExternalOutput")

    with tile.TileContext(nc) as tc:
        with (
            tc.tile_pool(name="consts", bufs=1) as cpool,
            tc.tile_pool(name="wb", bufs=3) as wpool,
            tc.tile_pool(name="crep", bufs=4) as rpool,
            tc.tile_pool(name="oh", bufs=4) as opool,
            tc.tile_pool(name="x", bufs=3) as xpool,
            tc.tile_pool(name="tree", bufs=2) as tpool,
            tc.tile_pool(name="feat", bufs=2) as fpool,
            tc.tile_pool(name="small", bufs=3) as spool,
            tc.tile_pool(name="psx", bufs=2, space="PSUM") as psx,
            tc.tile_pool(name="psa", bufs=2, space="PSUM") as psa,
            tc.tile_pool(name="pse", bufs=1, space="PSUM") as pse,
            tc.tile_pool(name="psh", bufs=1, space="PSUM") as psh,
        ):
            nc.gpsimd.load_library(library_config.mlp)

            # ---- constants (SWDGE: Pool idle during warmup) ----
            iotav = cpool.tile([128, 2], F32)
            nc.sync.dma_start(iotav[:], iotav_d[:])
            iota20 = cpool.tile([NPOS, 1], F32)
            nc.sync.dma_start(iota20[:], iota20_d[:])
            cemb = cpool.tile([128, 2, CE], BF16)
            nc.sync.dma_start(cemb[:], cemb_d[:])
            petab = cpool.tile([NPOS, PEM], BF16)
            nc.sync.dma_start(petab[:], petab_d[:])
            cwa = cpool.tile([60, 2, NA], FP8)
            nc.gpsimd.dma_start(cwa[:], cwa_d[:])
            cwb = cpool.tile([60, 2, NBW], FP8)
            nc.gpsimd.dma_start(cwb[:], cwb_d[:])
            ba = cpool.tile([NA, 1], F32)
            nc.gpsimd.dma_start(ba[:], ba_d[:])
            bb_t = cpool.tile([NB, 1], F32)
            nc.gpsimd.dma_start(bb_t[:], bb_d[:])
            whb = cpool.tile([128, HCH, HID], BF16)
            nc.gpsimd.dma_start(whb[:], whb_d[:])
            whca = cpool.tile([NA, HID], BF16)
            nc.gpsimd.dma_start(whca[:], whca_d[:])
            whcb = cpool.tile([NB, HID], BF16)
            nc.gpsimd.dma_start(whcb[:], whcb_d[:])
            whp = cpool.tile([PEM, HID], BF16)
            nc.gpsimd.dma_start(whp[:], whp_d[:])
            bh = cpool.tile([128, 2], F32)
            nc.gpsimd.dma_start(bh[:], bh_d[:])
            wc = cpool.tile([128, 2, NL], BF16)
            nc.gpsimd.dma_start(wc[:], wc_d[:])
            bc = cpool.tile([NL, 1], F32)
            nc.gpsimd.dma_start(bc[:], bc_d[:])
            iden = cpool.tile([NL, NL], F32)
            nc.gpsimd.dma_start(iden[:], id_d[:])
            wbidx = cpool.tile([128, BB, W // 16], I16)
            nc.sync.dma_start(wbidx[:], wbidx_d[:])
            prepall = cpool.tile([NPOS, BB, W], BF16)
            nc.gpsimd.dma_start(
                prepall[:], pflat_d[0:1, :].rearrange("o (b w) -> o b w", b=BB)
                .broadcast_to((NPOS, BB, W))
            )
            emtall = cpool.tile([128, BB, 2, NL], F32)

            def frontend(b):
                # ---- word_bert via dma_gather (feature-major transpose) ----
                wb = wpool.tile([128, HCH, W], BF16, tag="wb")
                nc.gpsimd.dma_gather(
                    wb[:], bert_d[b], wbidx[:, b, :],
                    num_idxs=W, num_idxs_reg=W, elem_size=H, transpose=True,
                )

                # ---- char one-hot + embed, half-stream pipelined ----
                # X physical [60, 2, NP]: DoubleRow k-tiles; logical tap row
                # 30j+e lives at partition e+30*(j%2), free-slot j//2.
                X = xpool.tile([60, 2, NP], FP8, tag="x")
                HP = NP // 2
                for hh in range(2):
                    csl = slice(b * NP + hh * HP, b * NP + (hh + 1) * HP)
                    crep = rpool.tile([128, HP], BF16, tag="crep")
                    nc.sync.dma_start(
                        crep[:], cflat_d[0:1, csl].broadcast_to((128, HP))
                    )
                    ohh = opool.tile([128, 2, HP], BF16, tag="oh")
                    for c in range(2):
                        nc.vector.tensor_scalar(
                            ohh[:, c, :], crep[:], iotav[:, c:c + 1], None, op0=ISEQ
                        )
                    for g in range(5):
                        px = psx.tile([CE, 512], F32, tag="px")
                        lsl = slice(512 * g, 512 * (g + 1))
                        xsl = slice(hh * HP + 512 * g, hh * HP + 512 * (g + 1))
                        for c in range(2):
                            nc.tensor.matmul(
                                px[:], cemb[:, c, :], ohh[:, c, lsl],
                                start=(c == 0), stop=(c == 1),
                            )
                        dst = X[0:CE, 0, xsl]
                        nc.scalar.copy(dst, px[:])
                    # tap replicas for this half (tap j = stream shifted by j);
                    # half 0 covers cols [0, HP-3), half 1 the rest.
                    lo = 0 if hh == 0 else HP - 3
                    hi = HP - 3 if hh == 0 else NP - 3
                    nc.sync.dma_start(X[CE:2 * CE, 0, lo:hi], X[0:CE, 0, lo + 1:hi + 1])
                    nc.gpsimd.dma_start(X[0:CE, 1, lo:hi], X[0:CE, 0, lo + 2:hi + 2])
                    nc.gpsimd.dma_start(X[CE:2 * CE, 1, lo:hi], X[0:CE, 0, lo + 3:hi + 3])
                    if hh == 1:
                        nc.sync.dma_start(
                            X[CE:2 * CE, 0, NP - 3:NP - 1], X[0:CE, 0, NP - 2:NP]
                        )
                        nc.gpsimd.dma_start(
                            X[0:CE, 1, NP - 3:NP - 2], X[0:CE, 0, NP - 1:NP]
                        )

                # ---- pos embedding ----
                ohp = spool.tile([NPOS, W], BF16, tag="ohp")
                nc.gpsimd.tensor_scalar(
                    ohp[:], prepall[:, b, :], iota20[:, 0:1], None, op0=ISEQ
                )
                psp = psx.tile([PEM, W], F32, tag="px")
                nc.tensor.matmul(psp[:], petab[:], ohp[:], start=True, stop=True)
                pf = spool.tile([PEM, W], BF16, tag="pf")
                nc.scalar.copy(pf[:], psp[:])
                return dict(wb=wb, X=X, pf=pf)

            def backend_conv(b, t):
                X = t["X"]
                X3 = X[:].rearrange("p i (w t) -> p i w t", t=TP)

                # ---- conv pass A: DR matmuls + tensor_reduce 16->1 per pair ----
                cfa = fpool.tile([128, W], BF16, tag="cfa")
                for v in range(4):  # 2 banks: words 64v..64v+63
                    pa = psa.tile([128, 64, 16], F32, tag="pa")
                    for h in range(4):
                        w0 = 64 * v + 16 * h
                        nc.tensor.matmul(
                            pa[:, 16 * h:16 * (h + 1), :],
                            cwa[:], X3[:, :, w0:w0 + 16, 0:16],
                            start=True, stop=True, perf_mode=DR,
                            skip_group_check=True,
                        )
                    if v == 3:
                        a16 = tpool.tile([128, 64, 16], BF16, tag="a16")
                        nc.scalar.copy(
                            a16[:].rearrange("p a b -> p (a b)"),
                            pa[:].rearrange("p a b -> p (a b)"),
                        )
                        a8 = tpool.tile([128, 64, 8], BF16, tag="a8")
                        nc.vector.tensor_tensor(
                            a8[:], a16[:, :, 0:8], a16[:, :, 8:16], op=MAX)
                        a4 = tpool.tile([128, 64, 4], BF16, tag="a4")
                        nc.vector.tensor_tensor(
                            a4[:], a8[:, :, 0:4], a8[:, :, 4:8], op=MAX)
                        a2 = tpool.tile([128, 64, 2], BF16, tag="a2")
                        nc.vector.tensor_tensor(
                            a2[:], a4[:, :, 0:2], a4[:, :, 2:4], op=MAX)
                        nc.vector.tensor_tensor(
                            cfa[:, 192:256].rearrange("p (w o) -> p w o", o=1),
                            a2[:, :, 0:1], a2[:, :, 1:2], op=MAX)
                    else:
                        nc.vector.tensor_reduce(
                            cfa[:, 64 * v:64 * (v + 1)], pa[:], axis=AXX, op=MAX,
                        )

                # ---- edge t=16 (valid rows 0:78 = k4|k2a; k3 rows ignored) ----
                pedge = pse.tile([128, 512], F32, tag="pe")
                nc.tensor.matmul(
                    pedge[:, 0:W], cwa[:], X3[:, :, :, 16],
                    start=True, stop=True, perf_mode=DR, skip_group_check=True,
                )
                nc.vector.tensor_tensor(
                    cfa[0:78, :], cfa[0:78, :], pedge[0:78, 0:W], op=MAX
                )
                cfa_r = fpool.tile([128, W], BF16, tag="cfar")
                nc.gpsimd.tensor_scalar(
                    cfa_r[:], cfa[:], ba[:, 0:1], 0.0, op0=ADD, op1=MAX
                )

                # ---- conv pass B (k2b): t-parity folded into rows ----
                # stationary col (tg, f): k2 taps at window taps (1+tg, 2+tg),
                # so column (w, even t) row (tg, f) = k2 filter f at position
                # t+tg. Even columns only => half the drain free-size.
                # psum row = 22*tg + f.
                bmax = tpool.tile([44, 2, 4, 32], BF16, tag="bmax")
                for r in range(2):
                    pb = psa.tile([44, 4, 32, 8], F32, tag="pa")
                    for h4 in range(4):
                        w0 = 128 * r + 32 * h4
                        nc.tensor.matmul(
                            pb[:, h4, :, :],
                            cwb[:, :, 0:44], X3[:, :, w0:w0 + 32, 0:16:2],
                            start=True, stop=True, perf_mode=DR,
                            skip_group_check=True,
                        )
                    nc.vector.tensor_reduce(
                        bmax[:, r, :, :], pb[:], axis=AXX, op=MAX,
                    )
                # edge t=16 (rows tg=0 valid; tg=1 = position 17, ignored)
                pbe = pedge[0:44, 256:512]
                nc.tensor.matmul(
                    pbe, cwb[:, :, 0:44], X3[:, :, :, 16],
                    start=True, stop=True, perf_mode=DR, skip_group_check=True,
                )
                # cross-partition fold: tg=1 rows -> partitions 0:22 via DMA
                btg1 = spool.tile([NB, W], BF16, tag="btg1")
                nc.sync.dma_start(
                    btg1[:], bmax[22:44, :, :, :].rearrange("p a b c -> p (a b c)")
                )
                cfb = fpool.tile([NB, W], BF16, tag="cfb")
                nc.vector.tensor_tensor(
                    cfb[:], bmax[0:22, :, :, :].rearrange("p a b c -> p (a b c)"),
                    btg1[:], op=MAX,
                )
                nc.vector.tensor_tensor(cfb[:], cfb[:], pbe[0:22, :], op=MAX)
                cfb_r = fpool.tile([NB, W], BF16, tag="cfbr")
                nc.gpsimd.tensor_scalar(
                    cfb_r[:], cfb[:], bb_t[:, 0:1], 0.0, op0=ADD, op1=MAX
                )
                t["cfa_r"] = cfa_r
                t["cfb_r"] = cfb_r
                t["pedge"] = pedge

            def backend_mlp(b, t):
                wb, pf = t["wb"], t["pf"]
                cfa_r, cfb_r = t["cfa_r"], t["cfb_r"]
                # ---- MLP1 ----
                hid_ps = psh.tile([128, 2, W], F32, tag="ph")
                for hc in range(2):
                    hsl = slice(128 * hc, 128 * (hc + 1))
                    for j in range(HCH):
                        nc.tensor.matmul(
                            hid_ps[:, hc, :], whb[:, j, hsl], wb[:, j, :],
                            start=(j == 0), stop=False, skip_group_check=True,
                        )
                    nc.tensor.matmul(
                        hid_ps[:, hc, :], whca[:, hsl], cfa_r[:],
                        start=False, stop=False, skip_group_check=True,
                    )
                    nc.tensor.matmul(
                        hid_ps[:, hc, :], whcb[:, hsl], cfb_r[:],
                        start=False, stop=False, skip_group_check=True,
                    )
                    nc.tensor.matmul(
                        hid_ps[:, hc, :], whp[:, hsl], pf[:],
                        start=False, stop=(hc == 1), skip_group_check=True,
                    )

                hid_sb = spool.tile([128, 2, W], BF16, tag="hid")
                for hc in range(2):
                    nc.scalar.activation(
                        hid_sb[:, hc, :], hid_ps[:, hc, :], RELU,
                        bias=bh[:, hc:hc + 1], scale=1.0,
                    )

                # ---- MLP2 + transpose + store (reuse edge psum bank) ----
                em_ps = t["pedge"]
                for hc in range(2):
                    nc.tensor.matmul(
                        em_ps[0:NL, 0:W], wc[:, hc, :], hid_sb[:, hc, :],
                        start=(hc == 0), stop=(hc == 1), skip_group_check=True,
                    )
                em_sb = spool.tile([NL, W], F32, tag="em")
                nc.scalar.activation(em_sb[:], em_ps[0:NL, 0:W], IDENT,
                                     bias=bc[:, 0:1], scale=1.0)
                for c in range(2):
                    nc.tensor.matmul(
                        em_ps[:, 384 + NL * c:384 + NL * (c + 1)],
                        em_sb[:, 128 * c:128 * (c + 1)], iden[:],
                        is_transpose=True,
                        start=(c == 0), stop=(c == 1), skip_group_check=True,
                    )
                nc.scalar.copy(emtall[:, b, :, :].rearrange("p c l -> p (c l)"),
                               em_ps[:, 384:384 + 2 * NL])

            tiles = {}
            for step in range(BB + 1):
                if step < BB:
                    tiles[step] = frontend(step)
                if step >= 1:
                    backend_conv(step - 1, tiles[step - 1])
                    backend_mlp(step - 1, tiles.pop(step - 1))
            nc.sync.dma_start(
                out_d[:].rearrange("b (c p) l -> p b c l", p=128), emtall[:]
            )

    lower_extended_insts(nc)
    _split_excess_waits(nc)
    return nc


def _prep_core(inputs, c):
    """Per-core input map (slicing / layout / dtype prep only)."""
    f32 = np.float32
    bf16 = ml_dtypes.bfloat16
    fp8 = ml_dtypes.float8_e4m3
    bsl = slice(c * BB, (c + 1) * BB)

    bert = np.zeros((BB, S + 1, H), bf16)
    bert[:, :S, :] = np.asarray(inputs["bert_hidden"][bsl], dtype=bf16)

    wid = np.asarray(inputs["word_ids"][bsl], dtype=np.int64)
    wbidx = np.zeros((128, BB, W // 16), np.int16)
    for b in range(BB):
        idx = np.full(W, S, np.int64)  # default -> zero row
        w_row = wid[b]
        first = np.ones(S, bool)
        first[1:] = w_row[1:] != w_row[:-1]
        pos = np.nonzero(first)[0]
        idx[w_row[pos]] = pos
        wrap = idx.reshape(16, 16).T.astype(np.int16)  # i -> (i%16, i//16)
        wbidx[:, b, :] = np.tile(wrap, (8, 1))

    cid = np.asarray(inputs["char_ids"][bsl], dtype=np.int64)
    cflat = np.zeros((1, BB * NP), bf16)
    for b in range(BB):
        tmpl = np.zeros((W, TP), np.int64)  # pads = char 0 (zero embedding)
        tmpl[:, 2:2 + LC] = cid[b]
        cflat[0, b * NP:(b + 1) * NP] = tmpl.reshape(NP).astype(bf16)

    pid = np.asarray(inputs["pos_ids"][bsl], dtype=np.int64)
    pflat = pid.reshape(1, BB * W).astype(bf16)

    iotav = np.empty((128, 2), f32)
    iotav[:, 0] = np.arange(128)
    iotav[:, 1] = np.arange(128, 256)
    iota20 = np.arange(NPOS, dtype=f32).reshape(NPOS, 1)
    cemb = np.ascontiguousarray(
        np.asarray(inputs["char_emb"], f32).reshape(2, 128, CE).transpose(1, 0, 2)
    ).astype(bf16)
    petab = np.asarray(inputs["pos_emb"], f32).astype(bf16)

    # unified 4-tap window weights: out pos t reads stream slots 20w+t+j,
    # tap j corresponds to char t-2+j.
    # A-row order: k4 f -> row f; k2a f(0:28) -> row 50+f; k3 f -> row 78+f.
    w2 = np.asarray(inputs["conv_w2"], f32)
    w3 = np.asarray(inputs["conv_w3"], f32)
    w4 = np.asarray(inputs["conv_w4"], f32)
    WU = np.zeros((120, NA), f32)  # [30j+e, row]
    for j in range(4):
        r = slice(30 * j, 30 * (j + 1))
        WU[r, 0:50] = w4[:, :, j].T
        if 1 <= j <= 2:
            WU[r, 50:78] = w2[0:28, :, j - 1].T
        if j >= 1:
            WU[r, 78:128] = w3[:, :, j - 1].T
    WUB = np.zeros((120, NBW), f32)  # col 22*tg + f: taps (1+tg, 2+tg)
    for tg in range(2):
        for j in (1, 2):
            WUB[30 * (j + tg):30 * (j + tg + 1), 22 * tg:22 * tg + NB] = \
                w2[28:50, :, j - 1].T
    # DR layout: contraction row 30j+e -> partition e+30*(j%2), k-tile j//2
    cwa = np.zeros((60, 2, NA), f32)
    cwb = np.zeros((60, 2, NBW), f32)
    for j in range(4):
        p = slice(30 * (j % 2), 30 * (j % 2) + 30)
        cwa[p, j // 2, :] = WU[30 * j:30 * (j + 1), :]
        cwb[p, j // 2, :] = WUB[30 * j:30 * (j + 1), :]
    cwa = cwa.astype(fp8)
    cwb = cwb.astype(fp8)

    b2_ = np.asarray(inputs["conv_b2"], f32)
    b3_ = np.asarray(inputs["conv_b3"], f32)
    b4_ = np.asarray(inputs["conv_b4"], f32)
    ba = np.zeros((NA, 1), f32)
    ba[0:50, 0] = b4_
    ba[50:78, 0] = b2_[0:28]
    ba[78:128, 0] = b3_
    bb_arr = b2_[28:50].reshape(NB, 1).astype(f32)

    wh = np.asarray(inputs["W_h"], f32)  # (943, 256)
    whb = np.ascontiguousarray(
        wh[:H].reshape(HCH, 128, HID).transpose(1, 0, 2)
    ).astype(bf16)
    # cf feature order in reference: [k2(50), k3(50), k4(50)] at rows 768+
    whca = np.zeros((NA, HID), f32)
    whca[0:50] = wh[768 + 100:768 + 150]      # k4
    whca[50:78] = wh[768:768 + 28]            # k2a
    whca[78:128] = wh[768 + 50:768 + 100]     # k3
    whca = whca.astype(bf16)
    whcb = np.ascontiguousarray(wh[768 + 28:768 + 50]).astype(bf16)
    whp = np.ascontiguousarray(wh[918:943]).astype(bf16)
    bh = np.asarray(inputs["b_h"], f32).reshape(2, 128).T.copy()
    wc = np.ascontiguousarray(
        np.asarray(inputs["W_c"], f32).reshape(2, 128, NL).transpose(1, 0, 2)
    ).astype(bf16)
    bc = np.asarray(inputs["b_c"], f32).reshape(NL, 1)
    ident = np.eye(NL, dtype=f32)

    return dict(
        bert=bert, wbidx=wbidx, cflat=cflat, pflat=pflat,
        iotav=iotav, iota20=iota20, cemb=cemb, petab=petab,
        cwa=cwa, cwb=cwb, ba=ba, bb=bb_arr,
        whb=whb, whca=whca, whcb=whcb, whp=whp,
        bh=bh, wc=wc, bc=bc, ident=ident,
    )


_NC_CACHE = {}


def kernel(**inputs) -> np.ndarray:
    if "nc" not in _NC_CACHE:
        _NC_CACHE["nc"] = build_nc()
    nc = _NC_CACHE["nc"]
    in_maps = [_prep_core(inputs, c) for c in range(N_CORES)]
    res = run_bass_kernel_spmd(nc, in_maps, list(range(N_CORES)))
    _NC_CACHE["last_result"] = res
    out = np.concatenate([res.results[c]["out"] for c in range(N_CORES)], axis=0)
    return out.astype(np.float32)



# revision 5
# speedup vs baseline: 1.4170x; 1.4170x over previous
"""BioBertNerAdvanced v2 kernel — 8 TRN2 cores, pure data parallel (8 batches/core).

Key structure vs v1:
  - word_bert scatter -> ONE dma_gather (transpose) per batch straight from
    DRAM bert rows (first-subword index per word, host-computed; missing
    words hit an appended zero row). Kills the one-hot scatter matmuls,
    their PSUM evac, and half the bert HBM traffic.
  - char conv on a unified 4-tap window: t=0..15 main positions + t=16 edge
    position. Pass A = 128 filter rows [k4(50)|k2a(28)|k3(78:128)] so the
    edge merge is one contiguous [0:78] slice; pass B = 22 rows (k2b)
    packed 2x at 64-partition pitch.
  - conv matmuls in fp8e4 DoubleRow (2 k-tiles of 60 partitions = 120-row
    contraction at 0.5 cyc/col).
  - max-over-time: pass A via DVE tensor_reduce straight off PSUM (single
    PSUM input is legal; tt with two PSUM inputs is not); pass B via
    ScalarE relu+bias evac then a Pool bf16 tt tree (GPSIMD cannot read
    PSUM). relu(max+b) == max(relu(+b)) makes the orders equivalent.
  - char embedding one-hot matmul in bf16 (DVE 4x is_equal), psum evacuated
    to fp8 X mostly on ScalarE.
  - two-stage software pipeline: frontend(b+1) [gather/one-hot/embed/X/pos]
    overlaps backend(b) [conv/drains/MLP]; bulk DMAs on SP/Act HWDGE
    queues; constants + preloads on SWDGE during warmup.
"""

import numpy as np
import ml_dtypes

import bass_rust
import concourse.bass as bass
import concourse.tile as tile
from concourse import mybir, library_config
from concourse.bass_utils import run_bass_kernel_spmd
from concourse.library_overlay import lower_extended_insts
from concourse.tile import ScopedClock


def _split_drain_and_barrier(self, tick_clock, wait_clock):
    """TileContext tail-drain emits one instruction with a sem wait per
    in-flight proc; walrus rejects >2 sync waits on one instruction. Spread
    the waits over SP nops (program order on SP makes the drain safe)."""
    gc = tick_clock.global_clock
    vals = list(gc)
    for i, v in enumerate(vals):
        if v > 0:
            part = [0] * len(vals)
            part[i] = v
            nop = self.nc.sync.nop()
            wait_clock.add_sem_waits(
                nop.ins, ScopedClock({None: bass_rust.VectorClock(part)})
            )
    drain_inst = self.nc.sync.drain()
    wait_clock.add_sem_waits(
        drain_inst.ins, ScopedClock({None: gc}), cur_clock=ScopedClock({None: gc})
    )
    self.nc.all_engine_barrier()
    assert self.sems is not None
    popped = self.nc._tile_sem_poison_stack.pop()
    assert popped is self._sem_poison
    self.nc.clear_and_free_semaphores(list(self.sems.allocated().values()))
    self.nc.all_engine_barrier()


tile.TileContext._drain_and_barrier = _split_drain_and_barrier

WAIT_LIMIT = 1


def _split_excess_waits(nc):
    """Walrus rejects instructions carrying more than WAIT_LIMIT sync waits.
    Spill the excess onto same-engine nops inserted immediately before the
    instruction (engine FIFO order makes the waits equivalent)."""
    for bb in nc.main_func.blocks:
        insts = bb.instructions
        out = []
        for ins in insts:
            si = ins.sync_info
            ow = list(si.on_wait) if si is not None and si.on_wait else []
            if len(ow) > WAIT_LIMIT:
                excess, keep = ow[:-WAIT_LIMIT], ow[-WAIT_LIMIT:]
                for i in range(0, len(excess), WAIT_LIMIT):
                    grp = excess[i:i + WAIT_LIMIT]
                    nop = nc.engines[ins.engine].nop(nofuse=True)
                    for bb2 in nc.main_func.blocks:
                        if bb2.instructions and bb2.instructions[-1] is nop.ins:
                            bb2.instructions.pop()
                            break
                    nop.ins.sync_info = mybir.SyncInfo(on_wait=grp, on_update=[])
                    out.append(nop.ins)
                si.on_wait = keep
            out.append(ins)
        insts[:] = out


# problem dims
B, S, H = 64, 512, 768
W, LC = 256, 16
CV, CE, NF = 256, 30, 50
NPOS, PEM = 20, 25
HID, NL = 256, 9
N_CORES = 8
BB = B // N_CORES
TP = 20                    # padded char pitch per word (2 + 16 + 2)
NP = W * TP                # char stream length per batch (5120)
HCH = 6                    # h-chunks of 128
NA = 128                   # pass-A rows: k4(0:50) k2a(50:78) k3(78:128)
NB = NF - 28               # pass-B filters (k2b, 22)
NBW = 64                   # B stationary width: (tg, f) pairs 2x22 padded to 64

F32 = mybir.dt.float32
BF16 = mybir.dt.bfloat16
FP8 = mybir.dt.float8e4
I16 = mybir.dt.int16

RELU = mybir.ActivationFunctionType.Relu
IDENT = mybir.ActivationFunctionType.Identity
MAX = mybir.AluOpType.max
ADD = mybir.AluOpType.add
ISEQ = mybir.AluOpType.is_equal
AXX = mybir.AxisListType.X
DR = mybir.MatmulPerfMode.DoubleRow


def build_nc():
    nc = bass.Bass("TRN2", target_bir_lowering=False, debug=False)

    bert_d = nc.dram_tensor("bert", [BB, S + 1, H], BF16, kind="ExternalInput")
    wbidx_d = nc.dram_tensor("wbidx", [128, BB, W // 16], I16, kind="ExternalInput")
    cflat_d = nc.dram_tensor("cflat", [1, BB * NP], BF16, kind="ExternalInput")
    pflat_d = nc.dram_tensor("pflat", [1, BB * W], BF16, kind="ExternalInput")
    iotav_d = nc.dram_tensor("iotav", [128, 2], F32, kind="ExternalInput")
    iota20_d = nc.dram_tensor("iota20", [NPOS, 1], F32, kind="ExternalInput")
    cemb_d = nc.dram_tensor("cemb", [128, 2, CE], BF16, kind="ExternalInput")
    petab_d = nc.dram_tensor("petab", [NPOS, PEM], BF16, kind="ExternalInput")
    cwa_d = nc.dram_tensor("cwa", [60, 2, NA], FP8, kind="ExternalInput")
    cwb_d = nc.dram_tensor("cwb", [60, 2, NBW], FP8, kind="ExternalInput")
    ba_d = nc.dram_tensor("ba", [NA, 1], F32, kind="ExternalInput")
    bb_d = nc.dram_tensor("bb", [NB, 1], F32, kind="ExternalInput")
    whb_d = nc.dram_tensor("whb", [128, HCH, HID], BF16, kind="ExternalInput")
    whca_d = nc.dram_tensor("whca", [NA, HID], BF16, kind="ExternalInput")
    whcb_d = nc.dram_tensor("whcb", [NB, HID], BF16, kind="ExternalInput")
    whp_d = nc.dram_tensor("whp", [PEM, HID], BF16, kind="ExternalInput")
    bh_d = nc.dram_tensor("bh", [128, 2], F32, kind="ExternalInput")
    wc_d = nc.dram_tensor("wc", [128, 2, NL], BF16, kind="ExternalInput")
    bc_d = nc.dram_tensor("bc", [NL, 1], F32, kind="ExternalInput")
    id_d = nc.dram_tensor("ident", [NL, NL], F32, kind="ExternalInput")
    out_d = nc.dram_tensor("out", [BB, W, NL], F32, kind="ExternalOutput")

    with tile.TileContext(nc) as tc:
        with (
            tc.tile_pool(name="consts", bufs=1) as cpool,
            tc.tile_pool(name="wb", bufs=3) as wpool,
            tc.tile_pool(name="crep", bufs=4) as rpool,
            tc.tile_pool(name="oh", bufs=4) as opool,
            tc.tile_pool(name="x", bufs=3) as xpool,
            tc.tile_pool(name="tree", bufs=2) as tpool,
            tc.tile_pool(name="feat", bufs=2) as fpool,
            tc.tile_pool(name="small", bufs=3) as spool,
            tc.tile_pool(name="psx", bufs=2, space="PSUM") as psx,
            tc.tile_pool(name="psa", bufs=2, space="PSUM") as psa,
            tc.tile_pool(name="pse", bufs=1, space="PSUM") as pse,
            tc.tile_pool(name="psh", bufs=1, space="PSUM") as psh,
        ):
            nc.gpsimd.load_library(library_config.mlp)

            # ---- constants (SWDGE: Pool idle during warmup) ----
            iotav = cpool.tile([128, 2], F32)
            nc.sync.dma_start(iotav[:], iotav_d[:])
            iota20 = cpool.tile([NPOS, 1], F32)
            nc.sync.dma_start(iota20[:], iota20_d[:])
            cemb = cpool.tile([128, 2, CE], BF16)
            nc.sync.dma_start(cemb[:], cemb_d[:])
            petab = cpool.tile([NPOS, PEM], BF16)
            nc.sync.dma_start(petab[:], petab_d[:])
            cwa = cpool.tile([60, 2, NA], FP8)
            nc.gpsimd.dma_start(cwa[:], cwa_d[:])
            cwb = cpool.tile([60, 2, NBW], FP8)
            nc.gpsimd.dma_start(cwb[:], cwb_d[:])
            ba = cpool.tile([NA, 1], F32)
            nc.gpsimd.dma_start(ba[:], ba_d[:])
            bb_t = cpool.tile([NB, 1], F32)
            nc.gpsimd.dma_start(bb_t[:], bb_d[:])
            whb = cpool.tile([128, HCH, HID], BF16)
            nc.gpsimd.dma_start(whb[:], whb_d[:])
            whca = cpool.tile([NA, HID], BF16)
            nc.gpsimd.dma_start(whca[:], whca_d[:])
            whcb = cpool.tile([NB, HID], BF16)
            nc.gpsimd.dma_start(whcb[:], whcb_d[:])
            whp = cpool.tile([PEM, HID], BF16)
            nc.gpsimd.dma_start(whp[:], whp_d[:])
            bh = cpool.tile([128, 2], F32)
            nc.gpsimd.dma_start(bh[:], bh_d[:])
            wc = cpool.tile([128, 2, NL], BF16)
            nc.gpsimd.dma_start(wc[:], wc_d[:])
            bc = cpool.tile([NL, 1], F32)
            nc.gpsimd.dma_start(bc[:], bc_d[:])
            iden = cpool.tile([NL, NL], F32)
            nc.gpsimd.dma_start(iden[:], id_d[:])
            wbidx = cpool.tile([128, BB, W // 16], I16)
            nc.sync.dma_start(wbidx[:], wbidx_d[:])
            prepall = cpool.tile([NPOS, BB, W], BF16)
            nc.gpsimd.dma_start(
                prepall[:], pflat_d[0:1, :].rearrange("o (b w) -> o b w", b=BB)
                .broadcast_to((NPOS, BB, W))
            )
            emtall = cpool.tile([128, BB, 2, NL], F32)

            def frontend(b):
                # ---- word_bert via dma_gather (feature-major transpose) ----
                wb = wpool.tile([128, HCH, W], BF16, tag="wb")
                nc.gpsimd.dma_gather(
                    wb[:], bert_d[b], wbidx[:, b, :],
                    num_idxs=W, num_idxs_reg=W, elem_size=H, transpose=True,
                )

                # ---- char one-hot + embed, half-stream pipelined ----
                # X physical [60, 2, NP]: DoubleRow k-tiles; logical tap row
                # 30j+e lives at partition e+30*(j%2), free-slot j//2.
                X = xpool.tile([60, 2, NP], FP8, tag="x")
                HP = NP // 2
                for hh in range(2):
                    csl = slice(b * NP + hh * HP, b * NP + (hh + 1) * HP)
                    crep = rpool.tile([128, HP], BF16, tag="crep")
                    nc.sync.dma_start(
                        crep[:], cflat_d[0:1, csl].broadcast_to((128, HP))
                    )
                    ohh = opool.tile([128, 2, HP], BF16, tag="oh")
                    for c in range(2):
                        nc.vector.tensor_scalar(
                            ohh[:, c, :], crep[:], iotav[:, c:c + 1], None, op0=ISEQ
                        )
                    for g in range(5):
                        px = psx.tile([CE, 512], F32, tag="px")
                        lsl = slice(512 * g, 512 * (g + 1))
                        xsl = slice(hh * HP + 512 * g, hh * HP + 512 * (g + 1))
                        for c in range(2):
                            nc.tensor.matmul(
                                px[:], cemb[:, c, :], ohh[:, c, lsl],
                                start=(c == 0), stop=(c == 1),
                            )
                        dst = X[0:CE, 0, xsl]
                        nc.scalar.copy(dst, px[:])
                    # tap replicas for this half (tap j = stream shifted by j);
                    # half 0 covers cols [0, HP-3), half 1 the rest.
                    lo = 0 if hh == 0 else HP - 3
                    hi = HP - 3 if hh == 0 else NP - 3
                    nc.sync.dma_start(X[CE:2 * CE, 0, lo:hi], X[0:CE, 0, lo + 1:hi + 1])
                    nc.scalar.dma_start(X[0:CE, 1, lo:hi], X[0:CE, 0, lo + 2:hi + 2])
                    nc.sync.dma_start(X[CE:2 * CE, 1, lo:hi], X[0:CE, 0, lo + 3:hi + 3])
                    if hh == 1:
                        nc.sync.dma_start(
                            X[CE:2 * CE, 0, NP - 3:NP - 1], X[0:CE, 0, NP - 2:NP]
                        )
                        nc.scalar.dma_start(
                            X[0:CE, 1, NP - 3:NP - 2], X[0:CE, 0, NP - 1:NP]
                        )

                # ---- pos embedding ----
                ohp = spool.tile([NPOS, W], BF16, tag="ohp")
                nc.vector.tensor_scalar(
                    ohp[:], prepall[:, b, :], iota20[:, 0:1], None, op0=ISEQ
                )
                psp = psx.tile([PEM, W], F32, tag="px")
                nc.tensor.matmul(psp[:], petab[:], ohp[:], start=True, stop=True)
                pf = spool.tile([PEM, W], BF16, tag="pf")
                nc.scalar.copy(pf[:], psp[:])
                return dict(wb=wb, X=X, pf=pf)

            def backend_conv(b, t):
                X = t["X"]
                X3 = X[:].rearrange("p i (w t) -> p i w t", t=TP)

                # ---- conv pass A: DR matmuls + tensor_reduce 16->1 per pair ----
                cfa = fpool.tile([128, W], BF16, tag="cfa")
                for v in range(4):  # 2 banks: words 64v..64v+63
                    pa = psa.tile([128, 64, 16], F32, tag="pa")
                    for h in range(4):
                        w0 = 64 * v + 16 * h
                        nc.tensor.matmul(
                            pa[:, 16 * h:16 * (h + 1), :],
                            cwa[:], X3[:, :, w0:w0 + 16, 0:16],
                            start=True, stop=True, perf_mode=DR,
                            skip_group_check=True,
                        )
                    if v == 3:
                        a16 = tpool.tile([128, 64, 16], BF16, tag="a16")
                        nc.scalar.copy(
                            a16[:].rearrange("p a b -> p (a b)"),
                            pa[:].rearrange("p a b -> p (a b)"),
                        )
                        a8 = tpool.tile([128, 64, 8], BF16, tag="a8")
                        nc.vector.tensor_tensor(
                            a8[:], a16[:, :, 0:8], a16[:, :, 8:16], op=MAX)
                        a4 = tpool.tile([128, 64, 4], BF16, tag="a4")
                        nc.vector.tensor_tensor(
                            a4[:], a8[:, :, 0:4], a8[:, :, 4:8], op=MAX)
                        a2 = tpool.tile([128, 64, 2], BF16, tag="a2")
                        nc.vector.tensor_tensor(
                            a2[:], a4[:, :, 0:2], a4[:, :, 2:4], op=MAX)
                        nc.vector.tensor_tensor(
                            cfa[:, 192:256].rearrange("p (w o) -> p w o", o=1),
                            a2[:, :, 0:1], a2[:, :, 1:2], op=MAX)
                    else:
                        nc.vector.tensor_reduce(
                            cfa[:, 64 * v:64 * (v + 1)], pa[:], axis=AXX, op=MAX,
                        )

                # ---- edge t=16 (valid rows 0:78 = k4|k2a; k3 rows ignored) ----
                pedge = pse.tile([128, 512], F32, tag="pe")
                nc.tensor.matmul(
                    pedge[:, 0:W], cwa[:], X3[:, :, :, 16],
                    start=True, stop=True, perf_mode=DR, skip_group_check=True,
                )
                nc.vector.tensor_tensor(
                    cfa[0:78, :], cfa[0:78, :], pedge[0:78, 0:W], op=MAX
                )
                cfa_r = fpool.tile([128, W], BF16, tag="cfar")
                nc.vector.tensor_scalar(
                    cfa_r[:], cfa[:], ba[:, 0:1], 0.0, op0=ADD, op1=MAX
                )

                # ---- conv pass B (k2b): t-parity folded into rows ----
                # stationary col (tg, f): k2 taps at window taps (1+tg, 2+tg),
                # so column (w, even t) row (tg, f) = k2 filter f at position
                # t+tg. Even columns only => half the drain free-size.
                # psum row = 22*tg + f.
                bmax = tpool.tile([44, 2, 4, 32], BF16, tag="bmax")
                for r in range(2):
                    pb = psa.tile([44, 4, 32, 8], F32, tag="pa")
                    for h4 in range(4):
                        w0 = 128 * r + 32 * h4
                        nc.tensor.matmul(
                            pb[:, h4, :, :],
                            cwb[:, :, 0:44], X3[:, :, w0:w0 + 32, 0:16:2],
                            start=True, stop=True, perf_mode=DR,
                            skip_group_check=True,
                        )
                    nc.vector.tensor_reduce(
                        bmax[:, r, :, :], pb[:], axis=AXX, op=MAX,
                    )
                # edge t=16 (rows tg=0 valid; tg=1 = position 17, ignored)
                pbe = pedge[0:44, 256:512]
                nc.tensor.matmul(
                    pbe, cwb[:, :, 0:44], X3[:, :, :, 16],
                    start=True, stop=True, perf_mode=DR, skip_group_check=True,
                )
                # cross-partition fold: tg=1 rows -> partitions 0:22 via DMA
                btg1 = spool.tile([NB, W], BF16, tag="btg1")
                nc.sync.dma_start(
                    btg1[:], bmax[22:44, :, :, :].rearrange("p a b c -> p (a b c)")
                )
                cfb = fpool.tile([NB, W], BF16, tag="cfb")
                nc.vector.tensor_tensor(
                    cfb[:], bmax[0:22, :, :, :].rearrange("p a b c -> p (a b c)"),
                    btg1[:], op=MAX,
                )
                nc.vector.tensor_tensor(cfb[:], cfb[:], pbe[0:22, :], op=MAX)
                cfb_r = fpool.tile([NB, W], BF16, tag="cfbr")
                nc.vector.tensor_scalar(
                    cfb_r[:], cfb[:], bb_t[:, 0:1], 0.0, op0=ADD, op1=MAX
                )
                t["cfa_r"] = cfa_r
                t["cfb_r"] = cfb_r
                t["pedge"] = pedge

            def backend_mlp(b, t):
                wb, pf = t["wb"], t["pf"]
                cfa_r, cfb_r = t["cfa_r"], t["cfb_r"]
                # ---- MLP1 ----
                hid_ps = psh.tile([128, 2, W], F32, tag="ph")
                for hc in range(2):
                    hsl = slice(128 * hc, 128 * (hc + 1))
                    for j in range(HCH):
                        nc.tensor.matmul(
                            hid_ps[:, hc, :], whb[:, j, hsl], wb[:, j, :],
                            start=(j == 0), stop=False, skip_group_check=True,
                        )
                    nc.tensor.matmul(
                        hid_ps[:, hc, :], whca[:, hsl], cfa_r[:],
                        start=False, stop=False, skip_group_check=True,
                    )
                    nc.tensor.matmul(
                        hid_ps[:, hc, :], whcb[:, hsl], cfb_r[:],
                        start=False, stop=False, skip_group_check=True,
                    )
                    nc.tensor.matmul(
                        hid_ps[:, hc, :], whp[:, hsl], pf[:],
                        start=False, stop=(hc == 1), skip_group_check=True,
                    )

                hid_sb = spool.tile([128, 2, W], BF16, tag="hid")
                for hc in range(2):
                    nc.scalar.activation(
                        hid_sb[:, hc, :], hid_ps[:, hc, :], RELU,
                        bias=bh[:, hc:hc + 1], scale=1.0,
                    )

                # ---- MLP2 + transpose + store (reuse edge psum bank) ----
                em_ps = t["pedge"]
                for hc in range(2):
                    nc.tensor.matmul(
                        em_ps[0:NL, 0:W], wc[:, hc, :], hid_sb[:, hc, :],
                        start=(hc == 0), stop=(hc == 1), skip_group_check=True,
                    )
                em_sb = spool.tile([NL, W], F32, tag="em")
                nc.scalar.activation(em_sb[:], em_ps[0:NL, 0:W], IDENT,
                                     bias=bc[:, 0:1], scale=1.0)
                for c in range(2):
                    nc.tensor.matmul(
                        em_ps[:, 384 + NL * c:384 + NL * (c + 1)],
                        em_sb[:, 128 * c:128 * (c + 1)], iden[:],
                        is_transpose=True,
                        start=(c == 0), stop=(c == 1), skip_group_check=True,
                    )
                nc.scalar.copy(emtall[:, b, :, :].rearrange("p c l -> p (c l)"),
                               em_ps[:, 384:384 + 2 * NL])

            tiles = {}
            for step in range(BB + 1):
                if step < BB:
                    tiles[step] = frontend(step)
                if step >= 1:
                    backend_conv(step - 1, tiles[step - 1])
                    backend_mlp(step - 1, tiles.pop(step - 1))
            nc.sync.dma_start(
                out_d[:].rearrange("b (c p) l -> p b c l", p=128), emtall[:]
            )

    lower_extended_insts(nc)
    _split_excess_waits(nc)
    return nc


def _prep_core(inputs, c):
    """Per-core input map (slicing / layout / dtype prep only)."""
    f32 = np.float32
    bf16 = ml_dtypes.bfloat16
    fp8 = ml_dtypes.float8_e4m3
    bsl = slice(c * BB, (c + 1) * BB)

    bert = np.zeros((BB, S + 1, H), bf16)
    bert[:, :S, :] = np.asarray(inputs["bert_hidden"][bsl], dtype=bf16)

    wid = np.asarray(inputs["word_ids"][bsl], dtype=np.int64)
    wbidx = np.zeros((128, BB, W // 16), np.int16)
    for b in range(BB):
        idx = np.full(W, S, np.int64)  # default -> zero row
        w_row = wid[b]
        first = np.ones(S, bool)
        first[1:] = w_row[1:] != w_row[:-1]
        pos = np.nonzero(first)[0]
        idx[w_row[pos]] = pos
        wrap = idx.reshape(16, 16).T.astype(np.int16)  # i -> (i%16, i//16)
        wbidx[:, b, :] = np.tile(wrap, (8, 1))

    cid = np.asarray(inputs["char_ids"][bsl], dtype=np.int64)
    cflat = np.zeros((1, BB * NP), bf16)
    for b in range(BB):
        tmpl = np.zeros((W, TP), np.int64)  # pads = char 0 (zero embedding)
        tmpl[:, 2:2 + LC] = cid[b]
        cflat[0, b * NP:(b + 1) * NP] = tmpl.reshape(NP).astype(bf16)

    pid = np.asarray(inputs["pos_ids"][bsl], dtype=np.int64)
    pflat = pid.reshape(1, BB * W).astype(bf16)

    iotav = np.empty((128, 2), f32)
    iotav[:, 0] = np.arange(128)
    iotav[:, 1] = np.arange(128, 256)
    iota20 = np.arange(NPOS, dtype=f32).reshape(NPOS, 1)
    cemb = np.ascontiguousarray(
        np.asarray(inputs["char_emb"], f32).reshape(2, 128, CE).transpose(1, 0, 2)
    ).astype(bf16)
    petab = np.asarray(inputs["pos_emb"], f32).astype(bf16)

    # unified 4-tap window weights: out pos t reads stream slots 20w+t+j,
    # tap j corresponds to char t-2+j.
    # A-row order: k4 f -> row f; k2a f(0:28) -> row 50+f; k3 f -> row 78+f.
    w2 = np.asarray(inputs["conv_w2"], f32)
    w3 = np.asarray(inputs["conv_w3"], f32)
    w4 = np.asarray(inputs["conv_w4"], f32)
    WU = np.zeros((120, NA), f32)  # [30j+e, row]
    for j in range(4):
        r = slice(30 * j, 30 * (j + 1))
        WU[r, 0:50] = w4[:, :, j].T
        if 1 <= j <= 2:
            WU[r, 50:78] = w2[0:28, :, j - 1].T
        if j >= 1:
            WU[r, 78:128] = w3[:, :, j - 1].T
    WUB = np.zeros((120, NBW), f32)  # col 22*tg + f: taps (1+tg, 2+tg)
    for tg in range(2):
        for j in (1, 2):
            WUB[30 * (j + tg):30 * (j + tg + 1), 22 * tg:22 * tg + NB] = \
                w2[28:50, :, j - 1].T
    # DR layout: contraction row 30j+e -> partition e+30*(j%2), k-tile j//2
    cwa = np.zeros((60, 2, NA), f32)
    cwb = np.zeros((60, 2, NBW), f32)
    for j in range(4):
        p = slice(30 * (j % 2), 30 * (j % 2) + 30)
        cwa[p, j // 2, :] = WU[30 * j:30 * (j + 1), :]
        cwb[p, j // 2, :] = WUB[30 * j:30 * (j + 1), :]
    cwa = cwa.astype(fp8)
    cwb = cwb.astype(fp8)

    b2_ = np.asarray(inputs["conv_b2"], f32)
    b3_ = np.asarray(inputs["conv_b3"], f32)
    b4_ = np.asarray(inputs["conv_b4"], f32)
    ba = np.zeros((NA, 1), f32)
    ba[0:50, 0] = b4_
    ba[50:78, 0] = b2_[0:28]
    ba[78:128, 0] = b3_
    bb_arr = b2_[28:50].reshape(NB, 1).astype(f32)

    wh = np.asarray(inputs["W_h"], f32)  # (943, 256)
    whb = np.ascontiguousarray(
        wh[:H].reshape(HCH, 128, HID).transpose(1, 0, 2)
    ).astype(bf16)
    # cf feature order in reference: [k2(50), k3(50), k4(50)] at rows 768+
    whca = np.zeros((NA, HID), f32)
    whca[0:50] = wh[768 + 100:768 + 150]      # k4
    whca[50:78] = wh[768:768 + 28]            # k2a
    whca[78:128] = wh[768 + 50:768 + 100]     # k3
    whca = whca.astype(bf16)
    whcb = np.ascontiguousarray(wh[768 + 28:768 + 50]).astype(bf16)
    whp = np.ascontiguousarray(wh[918:943]).astype(bf16)
    bh = np.asarray(inputs["b_h"], f32).reshape(2, 128).T.copy()
    wc = np.ascontiguousarray(
        np.asarray(inputs["W_c"], f32).reshape(2, 128, NL).transpose(1, 0, 2)
    ).astype(bf16)
    bc = np.asarray(inputs["b_c"], f32).reshape(NL, 1)
    ident = np.eye(NL, dtype=f32)

    return dict(
        bert=bert, wbidx=wbidx, cflat=cflat, pflat=pflat,
        iotav=iotav, iota20=iota20, cemb=cemb, petab=petab,
        cwa=cwa, cwb=cwb, ba=ba, bb=bb_arr,
        whb=whb, whca=whca, whcb=whcb, whp=whp,
        bh=bh, wc=wc, bc=bc, ident=ident,
    )


_NC_CACHE = {}


def kernel(**inputs) -> np.ndarray:
    if "nc" not in _NC_CACHE:
        _NC_CACHE["nc"] = build_nc()
    nc = _NC_CACHE["nc"]
    in_maps = [_prep_core(inputs, c) for c in range(N_CORES)]
    res = run_bass_kernel_spmd(nc, in_maps, list(range(N_CORES)))
    _NC_CACHE["last_result"] = res
    out = np.concatenate([res.results[c]["out"] for c in range(N_CORES)], axis=0)
    return out.astype(np.float32)

